# revision 14
# baseline (speedup 1.0000x reference)
"""YOLO-style detection decode (nms_detection) on 8 trn2 NeuronCores.

Data-parallel over batch (64 -> 8 images/core). The host repacks each
core's inputs into (cell, anchor)-major rows of 85 f32 — which is
exactly the raw per-anchor channel block [conf, x, y, w, h, cls x80] —
so the device needs NO transposes and every operand is a dense <=3D
access pattern:

  DRAM row r = cell*3 + a  (85 f32, 340B)

Rows are padded per scale to whole 128-row chunks; chunks are grouped
G at a time and stored [group][partition][G*85] so each group loads
with ONE DMA of 128 contiguous ~16KB packets (the original kernel was
DMA-descriptor-bound: 51k packets averaging 573B).

Device pipeline per group (no PE, no PSUM, one 128-lane row per
(cell,anchor)):
  - DVE : m  = reduce_max over the 80 class cols     [p, g, 80] f32
  - DVE : eq = (cls >= m) -> bf16 {0,1}  (exact: m is a member value)
  - DVE : eq *= iota (bf16 79-c); m2 = reduce_max(eq) = 79 - argmax
          (first-index ties win automatically: larger 79-c)
  - ACT : conf = Sigmoid(col0), ewh = Exp(cols 3:5)   (per-chunk
          grid offsets gx*t/416 and anchors/416 are shipped per
          (partition, chunk) so no per-scale special casing)
  - DVE : cx/cy fused stt, w/h = ewh*anchors, cls = 79 - m2,
          mask = (conf_logit > 0) * row   (one stt, 3D)
  - results accumulate in one SBUF-resident [128, 666*6] tile,
    flushed with a single 128x16KB DMA at the end.
"""

import os
from contextlib import ExitStack

import numpy as np

import concourse.bass as bass
import concourse.tile as tile
from concourse import bacc, mybir
from concourse.bass_utils import run_bass_kernel_spmd

N_CORES = 8
B = 64
B_PER = B // N_CORES
CASE = 416.0
SCALES = [("52", 52, 8.0), ("26", 26, 16.0), ("13", 13, 32.0)]
CHUNK = 128
ROW = 85           # f32 per (cell, anchor) row
G = int(os.environ.get("KGRP", "64"))
F32 = mybir.dt.float32
BF16 = mybir.dt.bfloat16
AX = mybir.AxisListType
OP = mybir.AluOpType
AF = mybir.ActivationFunctionType


def _cells(h):
    return B_PER * h * h


def _rows(h):
    return 3 * _cells(h)


def _nch(h):
    return (_rows(h) + CHUNK - 1) // CHUNK


NCH = {tag: _nch(h) for tag, h, _ in SCALES}     # 507, 127, 32
TOT_CH = sum(NCH.values())                        # 666
CH_OFF = {}
_c = 0
for _tag, _h, _t in SCALES:
    CH_OFF[_tag] = _c
    _c += NCH[_tag]

# groups: (tag, scale_idx, j0, gc, x_off_elems). The first groups are
# small so the DVE pipeline fills before the first full-size strip DMA
# completes.
_RAMP = [8, 32]
GROUPS = []
_off = 0
for _si, (_tag, _h, _t) in enumerate(SCALES):
    _j0 = 0
    _ri = 0 if _si == 0 else len(_RAMP)
    while _j0 < NCH[_tag]:
        _lim = _RAMP[_ri] if _ri < len(_RAMP) else G
        _ri += 1
        _gc = min(_lim, NCH[_tag] - _j0)
        GROUPS.append((_tag, _si, _j0, _gc, _off))
        _off += CHUNK * _gc * ROW
        _j0 += _gc
_GXY_OFF = _off
_off += CHUNK * TOT_CH * 2
_ANCH_OFF = _off
_off += CHUNK * TOT_CH * 2
_IOTA_OFF = _off
_off += CHUNK * 40                                # 80 bf16 packed as 40 f32
TOTAL_IN = _off
OUT_COLS = TOT_CH * 6


def _consts():
    """gxy / anch per (partition, global chunk): [128, TOT_CH, 2]."""
    import ml_dtypes
    gxy = np.zeros((CHUNK, TOT_CH, 2), np.float32)
    anch = np.zeros((CHUNK, TOT_CH, 2), np.float32)  # filled at pack time
    for si, (tag, h, t) in enumerate(SCALES):
        hw = h * h
        nr = _rows(h)
        nch = NCH[tag]
        r = np.arange(nch * CHUNK)
        cell = r // 3
        simg = cell % hw
        gx = (simg % h).astype(np.float64) * t / CASE
        gy = (simg // h).astype(np.float64) * t / CASE
        gx[r >= nr] = 0.0
        gy[r >= nr] = 0.0
        j0 = CH_OFF[tag]
        gxy[:, j0:j0 + nch, 0] = gx.reshape(nch, CHUNK).T
        gxy[:, j0:j0 + nch, 1] = gy.reshape(nch, CHUNK).T
    iota = np.zeros((CHUNK, 80), ml_dtypes.bfloat16)
    iota[:, :] = (79.0 - np.arange(80))[None, :]
    return gxy, iota.view(np.float32)


_GXY, _IOTA = _consts()


def _anch_pj(anchors):
    """[128, TOT_CH, 2] f32: anchors[a(r), d] / 416 per (p, chunk)."""
    anch = np.zeros((CHUNK, TOT_CH, 2), np.float32)
    for si, (tag, h, t) in enumerate(SCALES):
        nr = _rows(h)
        nch = NCH[tag]
        a416 = np.asarray(anchors[tag], np.float64) / CASE  # [3, 2]
        r = np.arange(nch * CHUNK)
        av = a416[r % 3]                                    # [nch*128, 2]
        av[r >= nr] = 0.0
        j0 = CH_OFF[tag]
        anch[:, j0:j0 + nch, :] = av.reshape(nch, CHUNK, 2) \
            .transpose(1, 0, 2).astype(np.float32)
    return anch


def build():
    nc = bacc.Bacc("TRN2", target_bir_lowering=False, debug=False,
                   num_devices=N_CORES)
    xin = nc.dram_tensor("xin", [TOTAL_IN], F32, kind="ExternalInput").ap()
    oX = nc.dram_tensor("out", [CHUNK, OUT_COLS], F32,
                        kind="ExternalOutput").ap()

    with tile.TileContext(nc) as tc:
        with ExitStack() as ctx:
            p_c = ctx.enter_context(tc.tile_pool(name="consts", bufs=1))
            p_in = ctx.enter_context(tc.tile_pool(name="inp", bufs=3))
            p_eq = ctx.enter_context(tc.tile_pool(name="eq", bufs=3))
            p_s = ctx.enter_context(tc.tile_pool(name="small", bufs=3))

            def load_const(name, cols, off):
                t_ = p_c.tile([CHUNK, cols], F32, tag=name)
                nc.sync.dma_start(
                    t_[:], xin[off:off + CHUNK * cols]
                    .rearrange("(p f) -> p f", p=CHUNK))
                return t_

            gxy_t = load_const("gxy", TOT_CH * 2, _GXY_OFF)
            anch_t = load_const("anch", TOT_CH * 2, _ANCH_OFF)
            iota_t = load_const("iota", 40, _IOTA_OFF)
            gxy_v = gxy_t[:].rearrange("p (j q) -> p j q", q=2)
            anch_v = anch_t[:].rearrange("p (j q) -> p j q", q=2)
            iota80 = iota_t[:].bitcast(BF16)                # [128, 80]

            out_t = p_c.tile([CHUNK, OUT_COLS], F32, tag="out_t")
            o_all = out_t[:].rearrange("p (ch s) -> p ch s", s=6)

            for tag, si, j0, gc, xoff in GROUPS:
                _, h, t = SCALES[si]
                k = float(t / CASE)
                jg = CH_OFF[tag] + j0                       # global chunk idx

                strip = p_in.tile([CHUNK, G * ROW], F32, tag="strip")
                nc.sync.dma_start(
                    strip[0:CHUNK, 0:gc * ROW],
                    xin[xoff:xoff + CHUNK * gc * ROW]
                    .rearrange("(p f) -> p f", p=CHUNK))
                sv = strip[:].rearrange("p (g c) -> p g c", g=G)[:, 0:gc]
                cls_ap = sv[:, :, 5:85]                     # [p, gc, 80]

                o_v = o_all[:, jg:jg + gc]                  # [p, gc, 6]

                # ACT ops first: their table loads + latency overlap the
                # fat DVE passes below, so the final mask stt never stalls
                nc.scalar.activation(o_v[:, :, 0:1].squeeze(2),
                                     sv[:, :, 0:1].squeeze(2), AF.Sigmoid)
                ewh = p_s.tile([CHUNK, G * 2], F32, tag="ewh")
                ewh_v = ewh[:].rearrange("p (g q) -> p g q", q=2)[:, 0:gc]
                nc.scalar.activation(ewh_v, sv[:, :, 3:5], AF.Exp)

                m = p_s.tile([CHUNK, G], F32, tag="m")
                m_v = m[:, 0:gc]
                nc.vector.tensor_reduce(m_v, cls_ap, axis=AX.X, op=OP.max)

                eq = p_eq.tile([CHUNK, G * 80], BF16, tag="eq")
                eq_v = eq[:].rearrange("p (g r) -> p g r", g=G)[:, 0:gc]
                m_b = m_v.unsqueeze(2).broadcast_to([CHUNK, gc, 80])
                nc.vector.tensor_tensor(eq_v, cls_ap, m_b, op=OP.is_ge)

                iota_b = iota80.unsqueeze(1).broadcast_to([CHUNK, gc, 80])
                nc.vector.tensor_tensor(eq_v, eq_v, iota_b, op=OP.mult)

                # second reduce as a tensor_tensor max tree: TT has a 2x
                # bf16 uop while tensor_reduce is 1x-only on DVE.
                w = 80
                if os.environ.get("KTREE", "1") == "1":
                    while w > 5:
                        hw_ = w // 2
                        nc.vector.tensor_tensor(
                            eq_v[:, :, 0:hw_], eq_v[:, :, 0:hw_],
                            eq_v[:, :, hw_:2 * hw_], op=OP.max)
                        w = hw_
                m2 = p_s.tile([CHUNK, G], BF16, tag="m2")
                m2_v = m2[:, 0:gc]
                nc.vector.tensor_reduce(m2_v, eq_v[:, :, 0:w], axis=AX.X,
                                        op=OP.max)

                nc.vector.scalar_tensor_tensor(
                    o_v[:, :, 1:3], sv[:, :, 1:3], k,
                    gxy_v[:, jg:jg + gc, :], op0=OP.mult, op1=OP.add)

                nc.vector.tensor_tensor(o_v[:, :, 3:5], ewh_v,
                                        anch_v[:, jg:jg + gc, :], op=OP.mult)

                nc.vector.tensor_scalar(o_v[:, :, 5:6].squeeze(2), m2_v,
                                        -1.0, 79.0, op0=OP.mult, op1=OP.add)

                conf_b = sv[:, :, 0:1].broadcast_to([CHUNK, gc, 6])
                nc.vector.scalar_tensor_tensor(
                    o_v, conf_b, 0.0, o_v, op0=OP.is_gt, op1=OP.mult)

                if j0 + gc == NCH[tag]:
                    # flush this scale's finished output columns so the
                    # store overlaps later scales' compute
                    c0 = CH_OFF[tag] * 6
                    c1 = (CH_OFF[tag] + NCH[tag]) * 6
                    nc.sync.dma_start(oX[:, c0:c1], out_t[:, c0:c1])
    nc.compile()
    return nc


_NC = None


def _get_nc():
    global _NC
    if _NC is None:
        _NC = build()
    return _NC


def _make_anch(anchors):
    return _anch_pj(anchors).reshape(CHUNK, TOT_CH * 2)


def _pack_core(xs, anch):
    """xs: {tag: [B_PER, 255, h, h] f32}; anch: [128, TOT_CH*2] f32."""
    parts = []
    for si, (tag, h, t) in enumerate(SCALES):
        hw = h * h
        nr = _rows(h)
        nch = NCH[tag]
        x = np.asarray(xs[tag]).reshape(B_PER, 255, hw)
        rows = np.zeros((nch * CHUNK, ROW), np.float32)
        rows[:nr] = x.transpose(0, 2, 1).reshape(nr, ROW)
        a = rows.reshape(nch, CHUNK, ROW)
        for gtag, _, j0, gc, _ in GROUPS:
            if gtag != tag:
                continue
            parts.append(np.ascontiguousarray(
                a[j0:j0 + gc].transpose(1, 0, 2)).ravel())
    parts.append(_GXY.ravel())
    parts.append(np.asarray(anch, np.float32).ravel())
    parts.append(_IOTA.ravel())
    out = np.concatenate(parts)
    assert out.size == TOTAL_IN and out.dtype == np.float32
    return out


def _unpack(res):
    """res: list of per-core {"out": [128, OUT_COLS]} -> [681408, 6]."""
    parts = []
    for tag, h, _ in SCALES[::-1]:               # output order: 13, 26, 52
        nr = _rows(h)
        nch = NCH[tag]
        c0 = CH_OFF[tag] * 6
        for i in range(N_CORES):
            o = res[i]["out"][:, c0:c0 + nch * 6]
            parts.append(o.reshape(CHUNK, nch, 6).transpose(1, 0, 2)
                         .reshape(nch * CHUNK, 6)[:nr])
    return np.concatenate(parts, axis=0)


def kernel(out13, out26, out52, anchors13, anchors26, anchors52):
    nc = _get_nc()
    xs_all = {"13": np.asarray(out13), "26": np.asarray(out26),
              "52": np.asarray(out52)}
    anchors = {"13": np.asarray(anchors13), "26": np.asarray(anchors26),
               "52": np.asarray(anchors52)}
    anch = _make_anch(anchors)

    in_maps = []
    for i in range(N_CORES):
        xs = {tag: xs_all[tag][i * B_PER:(i + 1) * B_PER]
              for tag, _, _ in SCALES}
        in_maps.append({"xin": _pack_core(xs, anch)})

    res = run_bass_kernel_spmd(nc, in_maps, list(range(N_CORES))).results
    return _unpack(res)
